# revision 14
# baseline (speedup 1.0000x reference)
"""Depthwise causal conv1d kernel for Trainium2 (8 NeuronCores, SPMD).

Problem: x [B=8, T=4096, C=512] f32, weight [C=512, K=4] f32.
out[b, t, c] = sum_k weight[c, k] * x[b, t - 3 + k, c]   (causal, zero-pad)

Strategy (v2):
  - Data-parallel over batch: core b handles x[b].
  - Host-side layout: channels-first x[b].T with 3 leading zero columns
    -> [C=512, T+3=4099], cast fp16 (halves HBM traffic, ~2^-11 element
    error, rel err ~1e-3 vs the 2e-2 gate).
  - The PE runs at 2.0 GHz here (0.5 ns/col), so 3 taps alone cost
    ~25 us — the PE cannot carry the whole conv under the ~23 us DMA
    stream. The work is split into two independent paths per channel
    chunk (4 chunks of 128 channels):
      * PE path (front FRONT[c] columns of each chunk): 4-tap
        accumulating diag-matmuls (512-col moving) into [128,1024] f32
        PSUM tiles; the PSUM->SBUF f16 drain is a plain activation
        Copy on the otherwise-idle Scalar engine (~1.15 us/KCol).
      * V path (back of each chunk): DVE computes the conv directly in
        SBUF f16: tensor_scalar (tap 1, odd offset -> 2x_2P mode) then
        three in-place scalar_tensor_tensor accumulations (taps 0,2 at
        2x_1P; tap 3 odd-aligned at 1x). ~2.9 us/KCol.
    Fronts [3072, 2560, 2560, 1536] balance PE (~19.5us) vs DVE
    (~18.5us) given arrival order (PE eats each chunk's front as it
    lands; DVE trails on the backs).
  - PE DVFS: the PE runs at ~1 ns/col for ~4.5 us after any idle gap
    before reaching 0.5 ns/col, and the ramp restarts on every gap. A
    warmup bridge of dummy matmuls (reading a raw, never-written SBUF
    scratch with no dependency edges) spans from the PE preamble end
    (~6.8 us) until the first x piece's DMA completion semaphore is
    visible (~12 us), in accumulation groups of 3 alternating PSUM
    banks.
  - DMA: x pieces on the Scalar DGE queue ([0,1027) of chunk0 first so
    the first semaphore fires ASAP, then the rest); all weights merged
    into ONE tensor wdc (diag stationary for chunk0 + f16 weight table
    + f32 table bitcast into f16 lanes) so a single early semaphore on
    the Sync queue gates both the first matmul and the first V pass.
    Output ships per chunk front/back region on the Sync queue; chunk3
    regions split across the Scalar and Sync queues so the final
    completion semaphores land in parallel.
  - Chunk 1-3 diag stationaries (4 taps) are expanded from the weight
    table on GpSimd, off the critical path.
"""

import numpy as np

B, T, C, K = 8, 4096, 512, 4
P = 128  # partitions
NCHUNK = C // P  # 4 channel chunks
TP = T + K - 1  # padded time = 4099
TJ = 512  # matmul moving width; one PSUM bank
NW = NCHUNK * K  # 16 (chunk, tap) columns in the weight table
# PE-path front width per chunk; remainder is the DVE (V) path.
# Balanced so both chains end ~33us: PE at ~1.73ns/Kcol-4tap from
# ~13.5us, DVE at ~2.7-3.5ns/Kcol from ~14.5us.
FRONT = [3584, 3072, 2560, 2560]
WDC_W = 4 * P + NW + 2 * NW  # 560: wd0 | wcol f16 | wcol32 as f16 pairs
NWARM_BIG = 6  # 512-wide warmup matmuls (~427ns each, denormal-slow)
NWARM_SMALL = 2  # 128-wide warmup matmuls, fine-grained
# landing just past the first x piece's worst-case semaphore arrival

_compiled = None


def _build():
    import contextlib

    import concourse.bacc as bacc
    import concourse.bass as bass
    import concourse.mybir as mybir
    from concourse.tile import TileContext

    f32 = mybir.dt.float32
    f16 = mybir.dt.float16
    nc = bacc.Bacc(enable_partition_id=False)

    wdc_d = nc.declare_dram_parameter("wdc", [P, WDC_W], f16, isOutput=False)
    xw_d = nc.declare_dram_parameter("xw", [P, NCHUNK * TP], f16, isOutput=False)
    out_d = nc.declare_dram_parameter("out", [C, T], f16, isOutput=True)

    ctx = contextlib.ExitStack()
    # Raw (non-Tile) SBUF scratch for the PE warmup: never written, so
    # the warmup matmuls have no dependencies at all and issue the
    # moment the PE preamble ends. Garbage input is fine — the PSUM
    # results are overwritten by the first real start=True matmul.
    scr = ctx.enter_context(nc.sbuf_tensor([P, TJ], f16))

    with TileContext(nc) as tc:
        with (
            tc.tile_pool(name="xpool", bufs=1) as xpool,
            tc.tile_pool(name="wpool", bufs=1) as wpool,
            tc.tile_pool(name="opool", bufs=4) as opool,
            tc.tile_pool(name="vpool", bufs=2) as vpool,
            tc.tile_pool(name="ppool", bufs=4, space="PSUM") as ppool,
        ):
            # single merged weight DMA, first on the Sync queue: its
            # completion semaphore (~11.2us) gates the first matmul
            # (wd0), the GpSimd expansions (wcol) and the first V pass
            # (wcol32) all at once.
            wdc = wpool.tile([P, WDC_W], f16, name="wdc", tag="wdc")
            nc.sync.dma_start(out=wdc, in_=wdc_d[:, :])
            wd0 = wdc[:, 0 : 4 * P]
            wcol_off = 4 * P
            w32 = wdc[:, wcol_off + NW : wcol_off + 3 * NW].bitcast(f32)

            # x loads get the Scalar DGE queue to themselves. chunk0 in
            # two pieces so its first PSUM tile can start ASAP.
            xts = []
            xt0 = xpool.tile([P, TP], f16, name="xt0", tag="xt0")
            # chunk0 in two pieces: A covers PSUM tiles 0-1 (first on
            # the queue -> fast semaphore ~10.4us, so the PE starts
            # during the DVFS ramp and runs dry exactly when B's
            # semaphore lands ~14.5us).
            h0 = 4 * TJ + K - 1  # 2051
            nc.scalar.dma_start(out=xt0[:, :h0], in_=xw_d[:, 0:h0])
            nc.scalar.dma_start(out=xt0[:, h0:], in_=xw_d[:, h0:TP])
            xts.append(xt0)
            for c in range(1, NCHUNK):
                xt = xpool.tile([P, TP], f16, name=f"xt{c}", tag=f"xt{c}")
                nc.scalar.dma_start(out=xt, in_=xw_d[:, c * TP : (c + 1) * TP])
                xts.append(xt)

            # expand wdc's weight table into per-chunk diag stationary
            # tiles (4 taps) on GpSimd for chunks 1..3
            wts = [wd0]
            for c in range(1, NCHUNK):
                wt = wpool.tile([P, K * P], f16, name=f"wd{c}", tag=f"wd{c}")
                for k in range(K):
                    idx = wcol_off + c * K + k
                    wsrc = bass.AP(
                        wdc.tensor, wdc.offset + idx, [[WDC_W, P], [0, P]]
                    )
                    nc.gpsimd.affine_select(
                        out=wt[:, k * P : (k + 1) * P],
                        in_=wsrc,
                        compare_op=mybir.AluOpType.is_equal,
                        fill=0.0,
                        base=0,
                        # iota[p, i] = p - i; == 0 on the diagonal
                        pattern=[[-1, P]],
                        channel_multiplier=1,
                    )
                wts.append(wt)

            # V-path tmp buffers, allocated once and reused by every span:
            # the resulting WAR chains force the Tile scheduler to keep the
            # DVE stream in span order (otherwise it hoists a later chunk's
            # first op, which waits on that chunk's DMA, ahead of ready
            # work -> ~2us head-of-line stall).
            vt0 = vpool.tile([P, 2048], f16, name="vt0", tag="vt0")
            vt1 = vpool.tile([P, 2048], f16, name="vt1", tag="vt1")

            # PE warmup bridge (see module docstring)
            ptw = ppool.tile([P, 2 * TJ], f32, name="ptw", tag="pt")
            for i in range(NWARM_BIG + NWARM_SMALL):
                w = TJ if i < NWARM_BIG else P
                half = (i // 3) % 2
                nc.tensor.matmul(
                    ptw[:, half * TJ : half * TJ + w],
                    scr[:, :P],
                    scr[:, :w],
                    start=(i % 3 == 0),
                    stop=(i % 3 == 2 or i == NWARM_BIG + NWARM_SMALL - 1),
                )

            for c in range(NCHUNK):
                xv = xts[c]
                wt = wts[c]
                front = FRONT[c]
                ot = opool.tile([P, T], f16, tag="ot")

                # --- PE path: 4-tap accumulating matmuls + SE drain ---
                last = c == NCHUNK - 1
                for base in range(0, front, 2 * TJ):
                    tw = min(2 * TJ, front - base)
                    pt = ppool.tile([P, 2 * TJ], f32, name="pt", tag="pt")
                    for h in range(0, tw, TJ):
                        j = base + h
                        mw = min(TJ, tw - h)
                        for k in range(K):
                            nc.tensor.matmul(
                                pt[:, h : h + mw],
                                wt[:, k * P : (k + 1) * P],
                                xv[:, j + k : j + k + mw],
                                start=(k == 0),
                                stop=(k == K - 1),
                            )
                    nc.scalar.copy(out=ot[:, base : base + tw], in_=pt[:, :tw])
                    if last:
                        # ship chunk3's front per-tile as drains land
                        nc.sync.dma_start(
                            out=out_d[c * P : (c + 1) * P, base : base + tw],
                            in_=ot[:, base : base + tw],
                        )
                if not last:
                    nc.sync.dma_start(
                        out=out_d[c * P : (c + 1) * P, 0:front], in_=ot[:, 0:front]
                    )

                # --- V path: DVE conv in SBUF f16 on the chunk back.
                # tensor_scalar runs at 4x mode (~0.48ns/col) and
                # tensor_tensor at 2x (~0.68ns/col); scalar_tensor_tensor
                # only has a 1x uop (~1.25ns/col), so build the conv from
                # TS multiplies into tmps + TT adds instead of STTs. ---
                s, e = front, T
                W = e - s
                o_sp = ot[:, s:e]
                t0, t1 = vt0[:, :W], vt1[:, :W]
                wv = lambda k: w32[:, c * K + k : c * K + k + 1]
                xk = lambda k: xv[:, s + k : e + k]
                nc.vector.tensor_scalar_mul(t0, xk(0), wv(0))
                nc.vector.tensor_scalar_mul(t1, xk(1), wv(1))
                nc.vector.tensor_add(o_sp, t0, t1)
                nc.vector.tensor_scalar_mul(t1, xk(2), wv(2))
                nc.vector.tensor_add(o_sp, o_sp, t1)
                nc.vector.tensor_scalar_mul(t0, xk(3), wv(3))
                nc.vector.tensor_add(o_sp, o_sp, t0)
                # back-region ships ride the GpSimd queue: the Sync
                # engine's trigger stream otherwise head-of-line blocks
                # early front ships behind a late back-ship wait
                nc.gpsimd.dma_start(
                    out=out_d[c * P : (c + 1) * P, s:e], in_=ot[:, s:e]
                )

    nc.compile()
    ctx.close()
    return nc


def _prep_inputs(x: np.ndarray, weight: np.ndarray):
    # wcol[p, chunk*K + k] = weight[chunk*P + p, k]
    wcol = np.ascontiguousarray(
        weight.reshape(NCHUNK, P, K).transpose(1, 0, 2).reshape(P, NW)
    )
    # chunk0's diag stationary prebuilt: wd0[p, k*P + p] = weight[p, k]
    wd0 = np.zeros((P, K * P), dtype=np.float16)
    for k in range(K):
        wd0[np.arange(P), k * P + np.arange(P)] = weight[:P, k].astype(np.float16)
    # merged weight tensor: wd0 | wcol f16 | wcol f32 bitcast to f16 lanes
    wdc = np.concatenate(
        [
            wd0,
            wcol.astype(np.float16),
            np.ascontiguousarray(wcol.astype(np.float32)).view(np.float16),
        ],
        axis=1,
    )
    assert wdc.shape == (P, WDC_W) and wdc.dtype == np.float16
    xs = []
    for b in range(B):
        xp = np.zeros((C, TP), dtype=np.float32)
        xp[:, K - 1 :] = x[b].T  # [512, 4099], 3 leading zeros
        xw = np.ascontiguousarray(
            xp.reshape(NCHUNK, P, TP).transpose(1, 0, 2).reshape(P, NCHUNK * TP)
        ).astype(np.float16)
        xs.append(xw)
    return xs, wdc


def _ensure_axon_hooks():
    """This image's antenv package lacks axon_hooks; synthesize it so a
    trace=True / BASS_TRACE run of run_bass_kernel_spmd can profile
    instead of crashing on import."""
    import sys
    import types

    if "antenv.axon_hooks" in sys.modules:
        return
    mod = types.ModuleType("antenv.axon_hooks")
    state = {"hook": None}
    mod.set_axon_ntff_profile_hook = lambda h: state.__setitem__("hook", h)
    mod.get_axon_ntff_profile_hook = lambda: state["hook"]
    sys.modules["antenv.axon_hooks"] = mod
    try:
        if "/root/.axon_site" not in sys.path:
            sys.path.insert(0, "/root/.axon_site")
        from trn_agent_boot.trn_boot import _ntff_profile_via_ctypes

        mod.set_axon_ntff_profile_hook(
            _ntff_profile_via_ctypes("/opt/axon/libaxon_pjrt.so")
        )
    except Exception:
        pass  # hook stays None; concourse degrades to no-trace


def kernel(x: np.ndarray, weight: np.ndarray) -> np.ndarray:
    global _compiled
    _ensure_axon_hooks()
    from concourse import bass_utils

    x = np.ascontiguousarray(x, dtype=np.float32)
    weight = np.ascontiguousarray(weight, dtype=np.float32)

    if _compiled is None:
        _compiled = _build()
    nc = _compiled

    xs, wdc = _prep_inputs(x, weight)
    in_maps = [{"xw": xs[b], "wdc": wdc} for b in range(B)]
    res = bass_utils.run_bass_kernel_spmd(nc, in_maps, core_ids=list(range(B)))

    out = np.empty((B, T, C), dtype=np.float32)
    for b in range(B):
        out[b] = np.asarray(res.results[b]["out"]).astype(np.float32).T
    return out


# revision 15
# speedup vs baseline: 1.0814x; 1.0814x over previous
"""Depthwise causal conv1d kernel for Trainium2 (8 NeuronCores, SPMD).

Problem: x [B=8, T=4096, C=512] f32, weight [C=512, K=4] f32.
out[b, t, c] = sum_k weight[c, k] * x[b, t - 3 + k, c]   (causal, zero-pad)

Strategy (v2):
  - Data-parallel over batch: core b handles x[b].
  - Host-side layout: channels-first x[b].T with 3 leading zero columns
    -> [C=512, T+3=4099], cast fp16 (halves HBM traffic, ~2^-11 element
    error, rel err ~1e-3 vs the 2e-2 gate).
  - The PE runs at 2.0 GHz here (0.5 ns/col), so 3 taps alone cost
    ~25 us — the PE cannot carry the whole conv under the ~23 us DMA
    stream. The work is split into two independent paths per channel
    chunk (4 chunks of 128 channels):
      * PE path (front FRONT[c] columns of each chunk): 4-tap
        accumulating diag-matmuls (512-col moving) into [128,1024] f32
        PSUM tiles; the PSUM->SBUF f16 drain is a plain activation
        Copy on the otherwise-idle Scalar engine (~1.15 us/KCol).
      * V path (back of each chunk): DVE computes the conv directly in
        SBUF f16: tensor_scalar (tap 1, odd offset -> 2x_2P mode) then
        three in-place scalar_tensor_tensor accumulations (taps 0,2 at
        2x_1P; tap 3 odd-aligned at 1x). ~2.9 us/KCol.
    Fronts [3072, 2560, 2560, 1536] balance PE (~19.5us) vs DVE
    (~18.5us) given arrival order (PE eats each chunk's front as it
    lands; DVE trails on the backs).
  - PE DVFS: the PE runs at ~1 ns/col for ~4.5 us after any idle gap
    before reaching 0.5 ns/col, and the ramp restarts on every gap. A
    warmup bridge of dummy matmuls (reading a raw, never-written SBUF
    scratch with no dependency edges) spans from the PE preamble end
    (~6.8 us) until the first x piece's DMA completion semaphore is
    visible (~12 us), in accumulation groups of 3 alternating PSUM
    banks.
  - DMA: x pieces on the Scalar DGE queue ([0,1027) of chunk0 first so
    the first semaphore fires ASAP, then the rest); all weights merged
    into ONE tensor wdc (diag stationary for chunk0 + f16 weight table
    + f32 table bitcast into f16 lanes) so a single early semaphore on
    the Sync queue gates both the first matmul and the first V pass.
    Output ships per chunk front/back region on the Sync queue; chunk3
    regions split across the Scalar and Sync queues so the final
    completion semaphores land in parallel.
  - Chunk 1-3 diag stationaries (4 taps) are expanded from the weight
    table on GpSimd, off the critical path.
"""

import numpy as np

B, T, C, K = 8, 4096, 512, 4
P = 128  # partitions
NCHUNK = C // P  # 4 channel chunks
TP = T + K - 1  # padded time = 4099
TJ = 512  # matmul moving width; one PSUM bank
NW = NCHUNK * K  # 16 (chunk, tap) columns in the weight table
# PE-path front width per chunk; remainder is the DVE (V) path.
# Balanced so both chains end ~33us: PE at ~1.73ns/Kcol-4tap from
# ~13.5us, DVE at ~2.7-3.5ns/Kcol from ~14.5us.
FRONT = [3584, 3072, 2560, 2304]
WDC_W = 4 * P + NW + 2 * NW  # 560: wd0 | wcol f16 | wcol32 as f16 pairs
NWARM_BIG = 9  # 512-wide warmup matmuls (~427ns each during the ramp)
NWARM_SMALL = 16  # 128-wide warmup matmuls, fine-grained
# landing just past the first x piece's worst-case semaphore arrival

_compiled = None


def _build():
    import contextlib

    import concourse.bacc as bacc
    import concourse.bass as bass
    import concourse.mybir as mybir
    from concourse.tile import TileContext

    f32 = mybir.dt.float32
    f16 = mybir.dt.float16
    nc = bacc.Bacc(enable_partition_id=False)

    wdc_d = nc.declare_dram_parameter("wdc", [P, WDC_W], f16, isOutput=False)
    xw_d = nc.declare_dram_parameter("xw", [P, NCHUNK * TP], f16, isOutput=False)
    out_d = nc.declare_dram_parameter("out", [C, T], f16, isOutput=True)

    ctx = contextlib.ExitStack()
    # Raw (non-Tile) SBUF scratch for the PE warmup: never written, so
    # the warmup matmuls have no dependencies at all and issue the
    # moment the PE preamble ends. Garbage input is fine — the PSUM
    # results are overwritten by the first real start=True matmul.
    scr = ctx.enter_context(nc.sbuf_tensor([P, TJ], f16))

    with TileContext(nc) as tc:
        with (
            tc.tile_pool(name="xpool", bufs=1) as xpool,
            tc.tile_pool(name="wpool", bufs=1) as wpool,
            tc.tile_pool(name="opool", bufs=4) as opool,
            tc.tile_pool(name="vpool", bufs=2) as vpool,
            tc.tile_pool(name="ppool", bufs=4, space="PSUM") as ppool,
        ):
            # single merged weight DMA, first on the Sync queue: its
            # completion semaphore (~11.2us) gates the first matmul
            # (wd0), the GpSimd expansions (wcol) and the first V pass
            # (wcol32) all at once.
            wdc = wpool.tile([P, WDC_W], f16, name="wdc", tag="wdc")
            nc.sync.dma_start(out=wdc, in_=wdc_d[:, :])
            wd0 = wdc[:, 0 : 4 * P]
            wcol_off = 4 * P
            w32 = wdc[:, wcol_off + NW : wcol_off + 3 * NW].bitcast(f32)

            # x loads get the Scalar DGE queue to themselves. chunk0 in
            # two pieces so its first PSUM tile can start ASAP.
            xts = []
            xt0 = xpool.tile([P, TP], f16, name="xt0", tag="xt0")
            # chunk0 in two pieces so its first PSUM tile starts ASAP
            h0 = 2 * TJ + K - 1  # 1027
            nc.scalar.dma_start(out=xt0[:, :h0], in_=xw_d[:, 0:h0])
            nc.scalar.dma_start(out=xt0[:, h0:], in_=xw_d[:, h0:TP])
            xts.append(xt0)
            for c in range(1, NCHUNK):
                xt = xpool.tile([P, TP], f16, name=f"xt{c}", tag=f"xt{c}")
                nc.scalar.dma_start(out=xt, in_=xw_d[:, c * TP : (c + 1) * TP])
                xts.append(xt)

            # expand wdc's weight table into per-chunk diag stationary
            # tiles (4 taps) on GpSimd for chunks 1..3
            wts = [wd0]
            for c in range(1, NCHUNK):
                wt = wpool.tile([P, K * P], f16, name=f"wd{c}", tag=f"wd{c}")
                for k in range(K):
                    idx = wcol_off + c * K + k
                    wsrc = bass.AP(
                        wdc.tensor, wdc.offset + idx, [[WDC_W, P], [0, P]]
                    )
                    nc.gpsimd.affine_select(
                        out=wt[:, k * P : (k + 1) * P],
                        in_=wsrc,
                        compare_op=mybir.AluOpType.is_equal,
                        fill=0.0,
                        base=0,
                        # iota[p, i] = p - i; == 0 on the diagonal
                        pattern=[[-1, P]],
                        channel_multiplier=1,
                    )
                wts.append(wt)

            # V-path tmp buffers, allocated once and reused by every span:
            # the resulting WAR chains force the Tile scheduler to keep the
            # DVE stream in span order (otherwise it hoists a later chunk's
            # first op, which waits on that chunk's DMA, ahead of ready
            # work -> ~2us head-of-line stall).
            vt0 = vpool.tile([P, 2048], f16, name="vt0", tag="vt0")
            vt1 = vpool.tile([P, 2048], f16, name="vt1", tag="vt1")

            # PE warmup bridge (see module docstring)
            ptw = ppool.tile([P, 2 * TJ], f32, name="ptw", tag="pt")
            for i in range(NWARM_BIG + NWARM_SMALL):
                w = TJ if i < NWARM_BIG else P
                half = (i // 3) % 2
                nc.tensor.matmul(
                    ptw[:, half * TJ : half * TJ + w],
                    scr[:, :P],
                    scr[:, :w],
                    start=(i % 3 == 0),
                    stop=(i % 3 == 2 or i == NWARM_BIG + NWARM_SMALL - 1),
                )

            for c in range(NCHUNK):
                xv = xts[c]
                wt = wts[c]
                front = FRONT[c]
                ot = opool.tile([P, T], f16, tag="ot")

                # --- PE path: 4-tap accumulating matmuls + SE drain ---
                last = c == NCHUNK - 1
                for base in range(0, front, 2 * TJ):
                    tw = min(2 * TJ, front - base)
                    pt = ppool.tile([P, 2 * TJ], f32, name="pt", tag="pt")
                    for h in range(0, tw, TJ):
                        j = base + h
                        mw = min(TJ, tw - h)
                        for k in range(K):
                            nc.tensor.matmul(
                                pt[:, h : h + mw],
                                wt[:, k * P : (k + 1) * P],
                                xv[:, j + k : j + k + mw],
                                start=(k == 0),
                                stop=(k == K - 1),
                            )
                    nc.scalar.copy(out=ot[:, base : base + tw], in_=pt[:, :tw])
                    if last:
                        # ship chunk3's front per-tile as drains land
                        nc.sync.dma_start(
                            out=out_d[c * P : (c + 1) * P, base : base + tw],
                            in_=ot[:, base : base + tw],
                        )
                if not last:
                    nc.sync.dma_start(
                        out=out_d[c * P : (c + 1) * P, 0:front], in_=ot[:, 0:front]
                    )

                # --- V path: DVE conv in SBUF f16 on the chunk back.
                # tensor_scalar runs at 4x mode (~0.48ns/col) and
                # tensor_tensor at 2x (~0.68ns/col); scalar_tensor_tensor
                # only has a 1x uop (~1.25ns/col), so build the conv from
                # TS multiplies into tmps + TT adds instead of STTs. ---
                s, e = front, T
                W = e - s
                o_sp = ot[:, s:e]
                t0, t1 = vt0[:, :W], vt1[:, :W]
                wv = lambda k: w32[:, c * K + k : c * K + k + 1]
                xk = lambda k: xv[:, s + k : e + k]
                nc.vector.tensor_scalar_mul(t0, xk(0), wv(0))
                nc.vector.tensor_scalar_mul(t1, xk(1), wv(1))
                nc.vector.tensor_add(o_sp, t0, t1)
                nc.vector.tensor_scalar_mul(t1, xk(2), wv(2))
                nc.vector.tensor_add(o_sp, o_sp, t1)
                nc.vector.tensor_scalar_mul(t0, xk(3), wv(3))
                nc.vector.tensor_add(o_sp, o_sp, t0)
                # back-region ships ride the GpSimd queue: the Sync
                # engine's trigger stream otherwise head-of-line blocks
                # early front ships behind a late back-ship wait
                nc.gpsimd.dma_start(
                    out=out_d[c * P : (c + 1) * P, s:e], in_=ot[:, s:e]
                )

    nc.compile()
    ctx.close()
    return nc


def _prep_inputs(x: np.ndarray, weight: np.ndarray):
    # wcol[p, chunk*K + k] = weight[chunk*P + p, k]
    wcol = np.ascontiguousarray(
        weight.reshape(NCHUNK, P, K).transpose(1, 0, 2).reshape(P, NW)
    )
    # chunk0's diag stationary prebuilt: wd0[p, k*P + p] = weight[p, k]
    wd0 = np.zeros((P, K * P), dtype=np.float16)
    for k in range(K):
        wd0[np.arange(P), k * P + np.arange(P)] = weight[:P, k].astype(np.float16)
    # merged weight tensor: wd0 | wcol f16 | wcol f32 bitcast to f16 lanes
    wdc = np.concatenate(
        [
            wd0,
            wcol.astype(np.float16),
            np.ascontiguousarray(wcol.astype(np.float32)).view(np.float16),
        ],
        axis=1,
    )
    assert wdc.shape == (P, WDC_W) and wdc.dtype == np.float16
    xs = []
    for b in range(B):
        xp = np.zeros((C, TP), dtype=np.float32)
        xp[:, K - 1 :] = x[b].T  # [512, 4099], 3 leading zeros
        xw = np.ascontiguousarray(
            xp.reshape(NCHUNK, P, TP).transpose(1, 0, 2).reshape(P, NCHUNK * TP)
        ).astype(np.float16)
        xs.append(xw)
    return xs, wdc


def _ensure_axon_hooks():
    """This image's antenv package lacks axon_hooks; synthesize it so a
    trace=True / BASS_TRACE run of run_bass_kernel_spmd can profile
    instead of crashing on import."""
    import sys
    import types

    if "antenv.axon_hooks" in sys.modules:
        return
    mod = types.ModuleType("antenv.axon_hooks")
    state = {"hook": None}
    mod.set_axon_ntff_profile_hook = lambda h: state.__setitem__("hook", h)
    mod.get_axon_ntff_profile_hook = lambda: state["hook"]
    sys.modules["antenv.axon_hooks"] = mod
    try:
        if "/root/.axon_site" not in sys.path:
            sys.path.insert(0, "/root/.axon_site")
        from trn_agent_boot.trn_boot import _ntff_profile_via_ctypes

        mod.set_axon_ntff_profile_hook(
            _ntff_profile_via_ctypes("/opt/axon/libaxon_pjrt.so")
        )
    except Exception:
        pass  # hook stays None; concourse degrades to no-trace


def kernel(x: np.ndarray, weight: np.ndarray) -> np.ndarray:
    global _compiled
    _ensure_axon_hooks()
    from concourse import bass_utils

    x = np.ascontiguousarray(x, dtype=np.float32)
    weight = np.ascontiguousarray(weight, dtype=np.float32)

    if _compiled is None:
        _compiled = _build()
    nc = _compiled

    xs, wdc = _prep_inputs(x, weight)
    in_maps = [{"xw": xs[b], "wdc": wdc} for b in range(B)]
    res = bass_utils.run_bass_kernel_spmd(nc, in_maps, core_ids=list(range(B)))

    out = np.empty((B, T, C), dtype=np.float32)
    for b in range(B):
        out[b] = np.asarray(res.results[b]["out"]).astype(np.float32).T
    return out
